# revision 10
# baseline (speedup 1.0000x reference)
"""Trainium2 Bass kernel for nn_Model_17789754540645 (dense transformer
attention block: qkv -> per-head softmax(q k^T * sqrt(hd)) v -> proj).

Sharding (8 cores): data-parallel over batch (2) x tensor-parallel over heads
(16 heads -> 4 per core). Each core computes qkv for its 4 heads, full
attention for those heads, and a partial proj output (row-sharded Wproj);
the host sums the 4 partials per batch and adds bproj.

v2: all-fp16/bf16 matmul data path (q/k/scores fp16 keeps argmax-critical
precision: measured 7e-3 end-to-end vs 2e-2 budget), q/k kept resident in
SBUF (no DRAM round-trip), weights resident, exact per-row max split across
DVE/GpSimd, wide [128,1024] exp and transpose-copy tiles, softmax recip
folded into the attn transpose via a diagonal matrix (D-trick), software
pipelining of transpose/AV one unit behind scores/softmax.

Self-contained: hardcodes shapes; only needs the container's concourse stack.
"""

import sys
import numpy as np

for _p in ("/opt/trn_rl_repo", "/opt/pypackages"):
    if _p not in sys.path:
        sys.path.append(_p)

import concourse.bass as bass
import concourse.tile as tile
from concourse import mybir
from concourse.masks import make_identity
from concourse.vector_clock import ScopedClock, VectorClock

F32 = mybir.dt.float32
F16 = mybir.dt.float16
BF16 = mybir.dt.bfloat16
AX = mybir.AxisListType
OP = mybir.AluOpType
ACTF = mybir.ActivationFunctionType

B, S_FULL, E, H, HD = 2, 2048, 2048, 16, 128
N_CORES = 8
HLOC_FULL = H // (N_CORES // B)  # 4 heads per core



# ---------------------------------------------------------------------------
# Walrus workaround: this container's walrus rejects >1 semaphore wait on
# several instruction encodings. Split extra waits onto single-wait NoOps.
# ---------------------------------------------------------------------------
_split_counter = [0]


def _split_multi_waits(nc, max_waits=1):
    n = 0
    for fn in nc.m.functions:
        for bb in fn.blocks:
            out, changed = [], False
            for inst in bb.instructions:
                si = inst.sync_info
                waits = list(si.on_wait) if (si and si.on_wait) else []
                if len(waits) > max_waits:
                    changed = True
                    extra, keep = waits[:-max_waits], waits[-max_waits:]
                    for w in extra:
                        _split_counter[0] += 1
                        nop = mybir.InstNoOp(
                            name=f"I-wsplit-{_split_counter[0]}", ins=[], outs=[]
                        )
                        nop.engine = inst.engine
                        nop.sync_info = mybir.SyncInfo(on_wait=[w], on_update=[])
                        out.append(nop)
                        n += 1
                    inst.sync_info = mybir.SyncInfo(
                        on_wait=keep,
                        on_update=list(si.on_update) if si.on_update else [],
                    )
                out.append(inst)
            if changed:
                bb.instructions = out
    return n


def _drain_and_barrier_split(self, tick_clock, wait_clock):
    """Replacement for TileContext._drain_and_barrier emitting <=1 wait per
    instruction (stock version puts every outstanding sem wait on one Drain,
    which this walrus rejects)."""
    gc = tick_clock.global_clock
    n = len(gc)
    active = [i for i in range(n) if gc[i] > 0]
    observed = ScopedClock({None: VectorClock([0] * n)})
    for i in active:
        vals = [gc[j] if j == i else 0 for j in range(n)]
        partial = ScopedClock({None: VectorClock(vals)})
        nop_inst = self.nc.sync.nop(nofuse=True)
        wait_clock.add_sem_waits(nop_inst.ins, partial, observed)
        observed.update_past(partial)
    drain_inst = self.nc.sync.drain()
    wait_clock.add_sem_waits(drain_inst.ins, ScopedClock({None: gc}), observed)

    self.nc.all_engine_barrier()
    assert self.sems is not None
    popped = self.nc._tile_sem_poison_stack.pop()
    assert popped is self._sem_poison
    self.nc.clear_and_free_semaphores(list(self.sems.allocated().values()))
    self.nc.all_engine_barrier()


tile.TileContext._drain_and_barrier = _drain_and_barrier_split


# ---------------------------------------------------------------------------
# Device program (SPMD - same program on all 8 cores, per-core inputs differ)
# ---------------------------------------------------------------------------

def build_program(S=S_FULL, HLOC=HLOC_FULL, has_bias=True):
    NEC = E // 128          # 16 e-chunks (contraction for qkv)
    NSB = S // 512          # s blocks of 512
    NST = S // 128          # s tiles of 128
    NFT = 2 * HLOC          # qk feature tiles of 128 (q0 k0 q1 k1 ...)
    NQB = S // 512          # q blocks of 512
    NKT = S // 128          # k tiles of 128
    NEB = E // 512          # output eo blocks
    VW = HLOC * 128         # v width (512 at full size)
    # score tiles per q-tile: widths (multiples of 512, up to 1024)
    SCW = []
    rem = S
    while rem > 0:
        w = 1024 if rem >= 1024 else rem
        SCW.append(w)
        rem -= w
    NSC = len(SCW)

    nc = bass.Bass()
    xt_p = nc.declare_dram_parameter("xt", [E, S], F16, isOutput=False)
    wqk_p = nc.declare_dram_parameter("wqk", [NFT, 128, E], F16, isOutput=False)
    wv_p = nc.declare_dram_parameter("wv", [E, VW], F16, isOutput=False)
    bqk_p = nc.declare_dram_parameter("bqk", [1, NFT * 128], F16, isOutput=False)
    bv_p = nc.declare_dram_parameter("bv", [1, VW], F16, isOutput=False)
    wp_p = nc.declare_dram_parameter("wp", [VW, E], F16, isOutput=False)
    y_p = nc.declare_dram_parameter("y", [S, E], F16, isOutput=True)

    with tile.TileContext(nc) as tc:
        from contextlib import ExitStack

        with ExitStack() as outer:
            const = outer.enter_context(tc.tile_pool(name="const", bufs=1))
            stats = outer.enter_context(tc.tile_pool(name="stats", bufs=56))
            qk_pool = outer.enter_context(tc.tile_pool(name="qk_pool", bufs=1))
            v_pool = outer.enter_context(tc.tile_pool(name="v_pool", bufs=1))
            oT_pool = outer.enter_context(tc.tile_pool(name="oT_pool", bufs=1))
            wp_pool = outer.enter_context(tc.tile_pool(name="wp_pool", bufs=1))

            ident = const.tile([128, 128], BF16)
            make_identity(nc, ident[:])
            ones_f32 = const.tile([1, 512], F32)
            nc.any.memset(ones_f32[:], 1.0)
            ones = const.tile([1, 512], F16)
            nc.vector.tensor_copy(ones[:], ones_f32[:])
            bqk_sb = const.tile([1, NFT * 128], F16)
            nc.sync.dma_start(bqk_sb[:], bqk_p[:])
            bv_sb = const.tile([1, VW], F16)
            nc.sync.dma_start(bv_sb[:], bv_p[:])

            qk_sb = [
                qk_pool.tile([128, S], F16, name=f"qk{f}", tag=f"qk{f}")
                for f in range(NFT)
            ]
            v_sb = [
                v_pool.tile([128, VW], BF16, name=f"vsb{st}", tag=f"vsb{st}")
                for st in range(NST)
            ]
            oT_sb = [
                oT_pool.tile([128, S], F16, name=f"ot{h}", tag=f"ot{h}")
                for h in range(HLOC)
            ]
            wp_sb = [
                wp_pool.tile([128, E], F16, name=f"wp{r}", tag=f"wp{r}")
                for r in range(HLOC)
            ]

            # ---------------- Phase 1: QKV ----------------
            with ExitStack() as ph1:
                xt_pool = ph1.enter_context(tc.tile_pool(name="xt_pool", bufs=32))
                wqk_pool = ph1.enter_context(tc.tile_pool(name="wqk_pool", bufs=1))
                wv_pool = ph1.enter_context(tc.tile_pool(name="wv_pool", bufs=1))
                psv = ph1.enter_context(tc.tile_pool(name="psv", bufs=4, space="PSUM"))
                psq = ph1.enter_context(tc.tile_pool(name="psq", bufs=4, space="PSUM"))

                wv_sb = [
                    wv_pool.tile([128, VW], F16, name=f"wvsb{c}", tag=f"wvsb{c}")
                    for c in range(NEC)
                ]
                wqk_sb = [
                    wqk_pool.tile([128, E], F16, name=f"wqk{f}", tag=f"wqk{f}")
                    for f in range(NFT)
                ]

                xts = {}

                def emit_xt_dmas(sb, with_wqk=False):
                    for c in range(NEC):
                        t = xt_pool.tile([128, 512], F16, name="xt_t", tag="xt_t")
                        if sb == 0:
                            nc.sync.dma_start(
                                t[:, 0:256],
                                xt_p[c * 128:(c + 1) * 128, 0:256],
                            )
                            nc.sync.dma_start(
                                t[:, 256:512],
                                xt_p[c * 128:(c + 1) * 128, 256:512],
                            )
                        else:
                            nc.sync.dma_start(
                                t[:],
                                xt_p[c * 128:(c + 1) * 128,
                                     sb * 512:(sb + 1) * 512],
                            )
                        xts[(sb, c)] = t
                        if with_wqk:
                            # interleave wqk in 512-col chunks so the first
                            # qk matmuls aren't gated on a 512KB transfer
                            for ft in range(NFT):
                                if c < 4:
                                    nc.sync.dma_start(
                                        wqk_sb[ft][:, c * 512:(c + 1) * 512],
                                        wqk_p[ft, :, c * 512:(c + 1) * 512],
                                    )

                emit_xt_dmas(0, with_wqk=True)
                for c in range(NEC):
                    nc.sync.dma_start(wv_sb[c][:], wv_p[c * 128:(c + 1) * 128, :])
                # phase-3 weights: queue behind phase-1 weights
                for r in range(HLOC):
                    nc.sync.dma_start(wp_sb[r][:], wp_p[r * 128:(r + 1) * 128, :])

                ncopy = [0]
                for sb in range(NSB):
                    if sb + 1 < NSB:
                        emit_xt_dmas(sb + 1)
                    # qk part first: psum_qk = wqk_tile.T @ x_blk (+ bqk)
                    for ft in range(NFT):
                        ps_qk = psq.tile([128, 512], F32, name="ps_qk", tag="ps_qk")
                        for c in range(NEC):
                            nc.tensor.matmul(
                                ps_qk[:],
                                wqk_sb[ft][:, c * 128:(c + 1) * 128],
                                xts[(sb, c)][:],
                                start=(c == 0),
                                stop=(not has_bias and c == NEC - 1),
                            )
                        if has_bias:
                            nc.tensor.matmul(
                                ps_qk[:],
                                bqk_sb[:1, ft * 128:(ft + 1) * 128],
                                ones[:1, :512],
                                start=False,
                                stop=True,
                            )
                        eng = nc.vector if (ncopy[0] % 2 == 0) else nc.scalar
                        if eng is nc.scalar:
                            nc.scalar.copy(
                                qk_sb[ft][:, sb * 512:(sb + 1) * 512], ps_qk[:]
                            )
                        else:
                            nc.vector.tensor_copy(
                                qk_sb[ft][:, sb * 512:(sb + 1) * 512], ps_qk[:]
                            )
                        ncopy[0] += 1

                    # v part last: keeps the PE busy over the phase boundary
                    # while the last block's qk copies drain.
                    ps_v = [
                        psv.tile([128, VW], F32, name="ps_v", tag="ps_v")
                        for _ in range(4)
                    ]
                    for c in range(NEC):
                        for st in range(4):
                            nc.tensor.matmul(
                                ps_v[st][:],
                                xts[(sb, c)][:, st * 128:(st + 1) * 128],
                                wv_sb[c][:],
                                start=(c == 0),
                                stop=(not has_bias and c == NEC - 1),
                            )
                    for st in range(4):
                        if has_bias:
                            nc.tensor.matmul(
                                ps_v[st][:],
                                ones[:1, :128],
                                bv_sb[:1, :],
                                start=False,
                                stop=True,
                            )
                        if st % 2 == 0:
                            nc.vector.tensor_copy(v_sb[sb * 4 + st][:], ps_v[st][:])
                        else:
                            nc.scalar.copy(v_sb[sb * 4 + st][:], ps_v[st][:])

            # -------- Phase 2+3: attention + proj, interleaved --------
            # v4: PE transposes into PSUM (power-frugal: the DMA-xbar variant
            # tripped the chip's activity throttle to a 50% util clamp), but
            # the pt->at copies move off DVE/Act onto plain DMAs (2KB
            # descriptors). Proj is interleaved per q-block; DVE/Act loads
            # balanced (maxes+small+3/4 normalize on DVE; exp+1/4 normalize+
            # fin+y copies on Act).
            with ExitStack() as ph2:
                attn_pool = ph2.enter_context(tc.tile_pool(name="attn_pool", bufs=6))
                attnT_pool = ph2.enter_context(tc.tile_pool(name="attnT_pool", bufs=8))
                y_pool = ph2.enter_context(tc.tile_pool(name="y_pool", bufs=6))
                # PSUM banks: pss 2x[128,1024]f32 (4) + pt 1x[128,1024]bf16
                # (1) + pso [128,512] (1) + psy 2x[128,512] (2) = 8
                pss = ph2.enter_context(tc.tile_pool(name="pss", bufs=2, space="PSUM"))
                pst = ph2.enter_context(tc.tile_pool(name="pst", bufs=1, space="PSUM"))
                pso = ph2.enter_context(tc.tile_pool(name="pso", bufs=1, space="PSUM"))
                psy = ph2.enter_context(tc.tile_pool(name="psy", bufs=2, space="PSUM"))

                units = [(h, qb) for qb in range(NQB) for h in range(HLOC)]
                nrm_ctr = [0]

                def emit_head_qt(unit, qt):
                    """scores + softmax for one q-tile; returns normalized attn."""
                    h, qb = unit
                    qh, kh = qk_sb[2 * h], qk_sb[2 * h + 1]
                    qti = qb * 4 + qt
                    sc = [
                        pss.tile([128, w], F32, name="ps_s", tag="ps_s")
                        for w in SCW
                    ]
                    off = 0
                    for i, w in enumerate(SCW):
                        for j in range(w // 512):
                            nc.tensor.matmul(
                                sc[i][:, j * 512:(j + 1) * 512],
                                qh[:, qti * 128:(qti + 1) * 128],
                                kh[:, off:off + 512],
                                start=True,
                                stop=True,
                            )
                            off += 512
                    mAB = stats.tile([128, NSC], F32, name="mAB", tag="mAB")
                    for i in range(NSC):
                        nc.vector.tensor_reduce(
                            mAB[:, i:i + 1], sc[i][:], axis=AX.X, op=OP.max
                        )
                    negm = stats.tile([128, 1], F32, name="negm", tag="negm")
                    nc.vector.tensor_reduce(
                        negm[:], mAB[:], axis=AX.X, op=OP.max, negate=True
                    )
                    attn_t = attn_pool.tile([128, S], BF16, name="attn_t", tag="attn_t")
                    sm = stats.tile([128, NSC], F32, name="sm", tag="sm")
                    off = 0
                    for i, w in enumerate(SCW):
                        nc.scalar.activation(
                            attn_t[:, off:off + w],
                            sc[i][:],
                            ACTF.Exp,
                            bias=negm[:],
                            scale=1.0,
                            accum_out=sm[:, i:i + 1],
                        )
                        off += w
                    sumx = stats.tile([128, 1], F32, name="sumx", tag="sumx")
                    nc.vector.tensor_reduce(sumx[:], sm[:], axis=AX.X, op=OP.add)
                    recip = stats.tile([128, 1], F32, name="recip", tag="recip")
                    nc.vector.reciprocal(recip[:], sumx[:])
                    # balance: every 4th normalize on Act, rest on DVE
                    if nrm_ctr[0] % 4 == 3:
                        nc.scalar.mul(attn_t[:], attn_t[:], recip[:])
                    else:
                        nc.vector.tensor_scalar_mul(attn_t[:], attn_t[:], recip[:])
                    nrm_ctr[0] += 1
                    return attn_t

                def tail_chunks(state):
                    """Closures: PE transposes (x diag(recip)) + split pt->at
                    copies + (lagged) AV, one kt-pair per chunk."""
                    h, qb, attns = state
                    nktp = NKT // 2
                    ps_o_box = [None]
                    at_tiles = [None] * nktp

                    def emit_av(ktp):
                        for half in range(2):
                            kt = 2 * ktp + half
                            nc.tensor.matmul(
                                ps_o_box[0][:],
                                v_sb[kt][:, h * 128:(h + 1) * 128],
                                at_tiles[ktp][:, half * 512:(half + 1) * 512],
                                start=(kt == 0),
                                stop=(kt == NKT - 1),
                            )

                    def mk_ktp(ktp):
                        def go():
                            if ktp == 0:
                                ps_o_box[0] = pso.tile(
                                    [128, 512], F32, name="ps_o", tag="ps_o"
                                )
                            pt = pst.tile([128, 1024], BF16, name="ps_t", tag="ps_t")
                            for half in range(2):
                                kt = 2 * ktp + half
                                for qt in range(4):
                                    nc.tensor.transpose(
                                        pt[:, half * 512 + qt * 128:
                                           half * 512 + (qt + 1) * 128],
                                        attns[qt][:, kt * 128:(kt + 1) * 128],
                                        ident[:],
                                    )
                            at = attnT_pool.tile([128, 1024], BF16, name="at", tag="at")
                            at_tiles[ktp] = at
                            # split copy: halves on DVE and Act in parallel
                            nc.vector.tensor_copy(at[:, 0:512], pt[:, 0:512])
                            nc.scalar.copy(at[:, 512:1024], pt[:, 512:1024])
                            if ktp > 0:
                                emit_av(ktp - 1)
                        return go

                    def fin():
                        emit_av(nktp - 1)
                        nc.scalar.copy(
                            oT_sb[h][:, qb * 512:(qb + 1) * 512], ps_o_box[0][:]
                        )

                    return [mk_ktp(k) for k in range(nktp)] + [fin]

                def proj_chunks(qb):
                    chunks = []

                    def mk(eb, qt):
                        def go():
                            qti = qb * 4 + qt
                            ps_y = psy.tile([128, 512], F32, name="ps_y", tag="ps_y")
                            for hh in range(HLOC):
                                nc.tensor.matmul(
                                    ps_y[:],
                                    oT_sb[hh][:, qti * 128:(qti + 1) * 128],
                                    wp_sb[hh][:, eb * 512:(eb + 1) * 512],
                                    start=(hh == 0),
                                    stop=(hh == HLOC - 1),
                                )
                            y_t = y_pool.tile([128, 512], F16, name="y_t", tag="y_t")
                            nc.scalar.copy(y_t[:], ps_y[:])
                            nc.sync.dma_start(
                                y_p[qti * 128:(qti + 1) * 128,
                                    eb * 512:(eb + 1) * 512],
                                y_t[:],
                            )
                        return go

                    for eb in range(NEB):
                        for qt in range(4):
                            chunks.append(mk(eb, qt))
                    return chunks

                pending = []
                for iu, u in enumerate(units):
                    h, qb = u
                    nq = 4
                    for qt in range(nq):
                        attn_t = emit_head_qt(u, qt)
                        if qt == 0:
                            attns = []
                        attns.append(attn_t)
                        take = (len(pending) + (nq - 1 - qt)) // (nq - qt)
                        for _ in range(take):
                            pending.pop(0)()
                    pending = pending + tail_chunks((h, qb, attns))
                    if h == HLOC - 1:
                        pending = pending + proj_chunks(qb)
                for ch in pending:
                    ch()

    _split_multi_waits(nc)
    return nc


# ---------------------------------------------------------------------------
# Host-side sharding / gather
# ---------------------------------------------------------------------------

def _prep_in_maps(query, Wqkv, bqkv, Wproj, S=S_FULL, HLOC=HLOC_FULL, n_cores=N_CORES):
    scale = np.float32(HD ** 0.5)
    groups = n_cores // B
    in_maps = []
    xt_cache = {}
    for c in range(n_cores):
        b, g = c // groups, c % groups
        heads = [g * HLOC + hh for hh in range(HLOC)]
        if b not in xt_cache:
            xt_cache[b] = np.ascontiguousarray(
                query[b][:S].T.astype(np.float16)
            )
        NFT = 2 * HLOC
        wqk = np.empty((NFT, 128, E), dtype=np.float16)
        bqk = np.empty((NFT * 128,), dtype=np.float32)
        wv = np.empty((E, HLOC * 128), dtype=np.float16)
        bv = np.empty((HLOC * 128,), dtype=np.float32)
        wp = np.empty((HLOC * 128, E), dtype=np.float16)
        for hh, hd_ in enumerate(heads):
            base = hd_ * (3 * HD)
            wq = Wqkv[base:base + HD, :] * scale          # [128, E]
            wk = Wqkv[base + HD:base + 2 * HD, :]
            wvh = Wqkv[base + 2 * HD:base + 3 * HD, :]
            # [E,128] -> chunked [128, E] layout: arr[p, c*128+j] = W.T[c*128+p, j]
            wqk[2 * hh] = (
                wq.T.reshape(E // 128, 128, HD).transpose(1, 0, 2).reshape(128, E)
            ).astype(np.float16)
            wqk[2 * hh + 1] = (
                wk.T.reshape(E // 128, 128, HD).transpose(1, 0, 2).reshape(128, E)
            ).astype(np.float16)
            bqk[2 * hh * 128:(2 * hh + 1) * 128] = bqkv[base:base + HD] * scale
            bqk[(2 * hh + 1) * 128:(2 * hh + 2) * 128] = bqkv[base + HD:base + 2 * HD]
            wv[:, hh * 128:(hh + 1) * 128] = wvh.T.astype(np.float16)
            bv[hh * 128:(hh + 1) * 128] = bqkv[base + 2 * HD:base + 3 * HD]
            wp[hh * 128:(hh + 1) * 128, :] = Wproj[:, hd_ * HD:(hd_ + 1) * HD].T.astype(
                np.float16
            )
        in_maps.append(
            {
                "xt": xt_cache[b],
                "wqk": np.ascontiguousarray(wqk),
                "wv": np.ascontiguousarray(wv),
                "bqk": bqk.reshape(1, -1).astype(np.float16),
                "bv": bv.reshape(1, -1).astype(np.float16),
                "wp": np.ascontiguousarray(wp),
            }
        )
    return in_maps


_CACHE = {}


def _get_program(S=S_FULL, HLOC=HLOC_FULL, has_bias=True):
    key = (S, HLOC, has_bias)
    if key not in _CACHE:
        _CACHE[key] = build_program(S, HLOC, has_bias=has_bias)
    return _CACHE[key]


def run(query, Wqkv, bqkv, Wproj, bproj, trace=False, S=S_FULL, HLOC=HLOC_FULL,
        n_cores=N_CORES):
    from concourse.bass_utils import run_bass_kernel_spmd

    has_bias = bool(np.any(bqkv))
    nc = _get_program(S, HLOC, has_bias=has_bias)
    in_maps = _prep_in_maps(query, Wqkv, bqkv, Wproj, S=S, HLOC=HLOC, n_cores=n_cores)
    res = run_bass_kernel_spmd(
        nc, in_maps, core_ids=list(range(n_cores)), trace=trace
    )
    groups = n_cores // B
    out = np.zeros((B, S, E), dtype=np.float32)
    for c in range(n_cores):
        out[c // groups] += res.results[c]["y"].astype(np.float32)
    out += bproj.astype(np.float32)
    return out, res


def kernel(**inputs):
    out, _ = run(
        np.asarray(inputs["query"], dtype=np.float32),
        np.asarray(inputs["Wqkv"], dtype=np.float32),
        np.asarray(inputs["bqkv"], dtype=np.float32),
        np.asarray(inputs["Wproj"], dtype=np.float32),
        np.asarray(inputs["bproj"], dtype=np.float32),
        trace=False,
    )
    return out



# revision 11
# speedup vs baseline: 1.1417x; 1.1417x over previous
"""Trainium2 Bass kernel for nn_Model_17789754540645 (dense transformer
attention block: qkv -> per-head softmax(q k^T * sqrt(hd)) v -> proj).

Sharding (8 cores): data-parallel over batch (2) x tensor-parallel over heads
(16 heads -> 4 per core). Each core computes qkv for its 4 heads, full
attention for those heads, and a partial proj output (row-sharded Wproj);
the host sums the 4 partials per batch and adds bproj.

v2: all-fp16/bf16 matmul data path (q/k/scores fp16 keeps argmax-critical
precision: measured 7e-3 end-to-end vs 2e-2 budget), q/k kept resident in
SBUF (no DRAM round-trip), weights resident, exact per-row max split across
DVE/GpSimd, wide [128,1024] exp and transpose-copy tiles, softmax recip
folded into the attn transpose via a diagonal matrix (D-trick), software
pipelining of transpose/AV one unit behind scores/softmax.

Self-contained: hardcodes shapes; only needs the container's concourse stack.
"""

import sys
import numpy as np

for _p in ("/opt/trn_rl_repo", "/opt/pypackages"):
    if _p not in sys.path:
        sys.path.append(_p)

import concourse.bass as bass
import concourse.tile as tile
from concourse import mybir
from concourse.masks import make_identity
from concourse.vector_clock import ScopedClock, VectorClock

F32 = mybir.dt.float32
F16 = mybir.dt.float16
BF16 = mybir.dt.bfloat16
AX = mybir.AxisListType
OP = mybir.AluOpType
ACTF = mybir.ActivationFunctionType

B, S_FULL, E, H, HD = 2, 2048, 2048, 16, 128
N_CORES = 8
HLOC_FULL = H // (N_CORES // B)  # 4 heads per core



# ---------------------------------------------------------------------------
# Walrus workaround: this container's walrus rejects >1 semaphore wait on
# several instruction encodings. Split extra waits onto single-wait NoOps.
# ---------------------------------------------------------------------------
_split_counter = [0]


def _split_multi_waits(nc, max_waits=1):
    n = 0
    for fn in nc.m.functions:
        for bb in fn.blocks:
            out, changed = [], False
            for inst in bb.instructions:
                si = inst.sync_info
                waits = list(si.on_wait) if (si and si.on_wait) else []
                if len(waits) > max_waits:
                    changed = True
                    extra, keep = waits[:-max_waits], waits[-max_waits:]
                    for w in extra:
                        _split_counter[0] += 1
                        nop = mybir.InstNoOp(
                            name=f"I-wsplit-{_split_counter[0]}", ins=[], outs=[]
                        )
                        nop.engine = inst.engine
                        nop.sync_info = mybir.SyncInfo(on_wait=[w], on_update=[])
                        out.append(nop)
                        n += 1
                    inst.sync_info = mybir.SyncInfo(
                        on_wait=keep,
                        on_update=list(si.on_update) if si.on_update else [],
                    )
                out.append(inst)
            if changed:
                bb.instructions = out
    return n


def _drain_and_barrier_split(self, tick_clock, wait_clock):
    """Replacement for TileContext._drain_and_barrier emitting <=1 wait per
    instruction (stock version puts every outstanding sem wait on one Drain,
    which this walrus rejects)."""
    gc = tick_clock.global_clock
    n = len(gc)
    active = [i for i in range(n) if gc[i] > 0]
    observed = ScopedClock({None: VectorClock([0] * n)})
    for i in active:
        vals = [gc[j] if j == i else 0 for j in range(n)]
        partial = ScopedClock({None: VectorClock(vals)})
        nop_inst = self.nc.sync.nop(nofuse=True)
        wait_clock.add_sem_waits(nop_inst.ins, partial, observed)
        observed.update_past(partial)
    drain_inst = self.nc.sync.drain()
    wait_clock.add_sem_waits(drain_inst.ins, ScopedClock({None: gc}), observed)

    self.nc.all_engine_barrier()
    assert self.sems is not None
    popped = self.nc._tile_sem_poison_stack.pop()
    assert popped is self._sem_poison
    self.nc.clear_and_free_semaphores(list(self.sems.allocated().values()))
    self.nc.all_engine_barrier()


tile.TileContext._drain_and_barrier = _drain_and_barrier_split


# ---------------------------------------------------------------------------
# Device program (SPMD - same program on all 8 cores, per-core inputs differ)
# ---------------------------------------------------------------------------

def build_program(S=S_FULL, HLOC=HLOC_FULL, has_bias=True):
    NEC = E // 128          # 16 e-chunks (contraction for qkv)
    NSB = S // 512          # s blocks of 512
    NST = S // 128          # s tiles of 128
    NFT = 2 * HLOC          # qk feature tiles of 128 (q0 k0 q1 k1 ...)
    NQB = S // 512          # q blocks of 512
    NKT = S // 128          # k tiles of 128
    NEB = E // 512          # output eo blocks
    VW = HLOC * 128         # v width (512 at full size)
    # score tiles per q-tile: widths (multiples of 512, up to 1024)
    SCW = []
    rem = S
    while rem > 0:
        w = 1024 if rem >= 1024 else rem
        SCW.append(w)
        rem -= w
    NSC = len(SCW)

    nc = bass.Bass()
    xt_p = nc.declare_dram_parameter("xt", [E, S], F16, isOutput=False)
    wqk_p = nc.declare_dram_parameter("wqk", [NFT, 128, E], F16, isOutput=False)
    wv_p = nc.declare_dram_parameter("wv", [E, VW], F16, isOutput=False)
    bqk_p = nc.declare_dram_parameter("bqk", [1, NFT * 128], F16, isOutput=False)
    bv_p = nc.declare_dram_parameter("bv", [1, VW], F16, isOutput=False)
    wp_p = nc.declare_dram_parameter("wp", [VW, E], F16, isOutput=False)
    y_p = nc.declare_dram_parameter("y", [S, E], F16, isOutput=True)

    with tile.TileContext(nc) as tc:
        from contextlib import ExitStack

        with ExitStack() as outer:
            const = outer.enter_context(tc.tile_pool(name="const", bufs=1))
            stats = outer.enter_context(tc.tile_pool(name="stats", bufs=56))
            qk_pool = outer.enter_context(tc.tile_pool(name="qk_pool", bufs=1))
            v_pool = outer.enter_context(tc.tile_pool(name="v_pool", bufs=1))
            oT_pool = outer.enter_context(tc.tile_pool(name="oT_pool", bufs=1))
            wp_pool = outer.enter_context(tc.tile_pool(name="wp_pool", bufs=1))

            ident = const.tile([128, 128], BF16)
            make_identity(nc, ident[:])
            ones_f32 = const.tile([1, 512], F32)
            nc.any.memset(ones_f32[:], 1.0)
            ones = const.tile([1, 512], F16)
            nc.vector.tensor_copy(ones[:], ones_f32[:])
            bqk_sb = const.tile([1, NFT * 128], F16)
            nc.sync.dma_start(bqk_sb[:], bqk_p[:])
            bv_sb = const.tile([1, VW], F16)
            nc.sync.dma_start(bv_sb[:], bv_p[:])

            qk_sb = [
                qk_pool.tile([128, S], F16, name=f"qk{f}", tag=f"qk{f}")
                for f in range(NFT)
            ]
            v_sb = [
                v_pool.tile([128, VW], BF16, name=f"vsb{st}", tag=f"vsb{st}")
                for st in range(NST)
            ]
            oT_sb = [
                oT_pool.tile([128, S], F16, name=f"ot{h}", tag=f"ot{h}")
                for h in range(HLOC)
            ]
            wp_sb = [
                wp_pool.tile([128, E], F16, name=f"wp{r}", tag=f"wp{r}")
                for r in range(HLOC)
            ]

            # ---------------- Phase 1: QKV ----------------
            with ExitStack() as ph1:
                xt_pool = ph1.enter_context(tc.tile_pool(name="xt_pool", bufs=32))
                wqk_pool = ph1.enter_context(tc.tile_pool(name="wqk_pool", bufs=1))
                wv_pool = ph1.enter_context(tc.tile_pool(name="wv_pool", bufs=1))
                psv = ph1.enter_context(tc.tile_pool(name="psv", bufs=4, space="PSUM"))
                psq = ph1.enter_context(tc.tile_pool(name="psq", bufs=4, space="PSUM"))

                wv_sb = [
                    wv_pool.tile([128, VW], F16, name=f"wvsb{c}", tag=f"wvsb{c}")
                    for c in range(NEC)
                ]
                wqk_sb = [
                    wqk_pool.tile([128, E], F16, name=f"wqk{f}", tag=f"wqk{f}")
                    for f in range(NFT)
                ]

                xts = {}

                def emit_xt_dmas(sb, with_wqk=False):
                    for c in range(NEC):
                        t = xt_pool.tile([128, 512], F16, name="xt_t", tag="xt_t")
                        if sb == 0:
                            nc.sync.dma_start(
                                t[:, 0:256],
                                xt_p[c * 128:(c + 1) * 128, 0:256],
                            )
                            nc.sync.dma_start(
                                t[:, 256:512],
                                xt_p[c * 128:(c + 1) * 128, 256:512],
                            )
                        else:
                            nc.sync.dma_start(
                                t[:],
                                xt_p[c * 128:(c + 1) * 128,
                                     sb * 512:(sb + 1) * 512],
                            )
                        xts[(sb, c)] = t
                        if with_wqk:
                            # interleave wqk in 512-col chunks so the first
                            # qk matmuls aren't gated on a 512KB transfer
                            for ft in range(NFT):
                                if c < 4:
                                    nc.sync.dma_start(
                                        wqk_sb[ft][:, c * 512:(c + 1) * 512],
                                        wqk_p[ft, :, c * 512:(c + 1) * 512],
                                    )

                emit_xt_dmas(0, with_wqk=True)
                for c in range(NEC):
                    nc.sync.dma_start(wv_sb[c][:], wv_p[c * 128:(c + 1) * 128, :])
                # phase-3 weights: queue behind phase-1 weights
                for r in range(HLOC):
                    nc.sync.dma_start(wp_sb[r][:], wp_p[r * 128:(r + 1) * 128, :])

                ncopy = [0]
                for sb in range(NSB):
                    if sb + 1 < NSB:
                        emit_xt_dmas(sb + 1)
                    # qk part first: psum_qk = wqk_tile.T @ x_blk (+ bqk)
                    for ft in range(NFT):
                        ps_qk = psq.tile([128, 512], F32, name="ps_qk", tag="ps_qk")
                        for c in range(NEC):
                            nc.tensor.matmul(
                                ps_qk[:],
                                wqk_sb[ft][:, c * 128:(c + 1) * 128],
                                xts[(sb, c)][:],
                                start=(c == 0),
                                stop=(not has_bias and c == NEC - 1),
                            )
                        if has_bias:
                            nc.tensor.matmul(
                                ps_qk[:],
                                bqk_sb[:1, ft * 128:(ft + 1) * 128],
                                ones[:1, :512],
                                start=False,
                                stop=True,
                            )
                        eng = nc.vector if (ncopy[0] % 2 == 0) else nc.scalar
                        if eng is nc.scalar:
                            nc.scalar.copy(
                                qk_sb[ft][:, sb * 512:(sb + 1) * 512], ps_qk[:]
                            )
                        else:
                            nc.vector.tensor_copy(
                                qk_sb[ft][:, sb * 512:(sb + 1) * 512], ps_qk[:]
                            )
                        ncopy[0] += 1

                    # v part last: keeps the PE busy over the phase boundary
                    # while the last block's qk copies drain.
                    ps_v = [
                        psv.tile([128, VW], F32, name="ps_v", tag="ps_v")
                        for _ in range(4)
                    ]
                    for c in range(NEC):
                        for st in range(4):
                            nc.tensor.matmul(
                                ps_v[st][:],
                                xts[(sb, c)][:, st * 128:(st + 1) * 128],
                                wv_sb[c][:],
                                start=(c == 0),
                                stop=(not has_bias and c == NEC - 1),
                            )
                    for st in range(4):
                        if has_bias:
                            nc.tensor.matmul(
                                ps_v[st][:],
                                ones[:1, :128],
                                bv_sb[:1, :],
                                start=False,
                                stop=True,
                            )
                        if st % 2 == 0:
                            nc.vector.tensor_copy(v_sb[sb * 4 + st][:], ps_v[st][:])
                        else:
                            nc.scalar.copy(v_sb[sb * 4 + st][:], ps_v[st][:])

            # -------- Phase 2+3: attention + proj, interleaved --------
            with ExitStack() as ph2:
                attn_pool = ph2.enter_context(tc.tile_pool(name="attn_pool", bufs=10))
                attnT_pool = ph2.enter_context(tc.tile_pool(name="attnT_pool", bufs=8))
                # PSUM: pss 3 x [128,1024]f32 (6 banks) + pst 1 x [128,1024]bf16
                # (1 bank) + pso 1 x [128,512]f32 = 8
                pss = ph2.enter_context(tc.tile_pool(name="pss", bufs=3, space="PSUM"))
                pst = ph2.enter_context(tc.tile_pool(name="pst", bufs=1, space="PSUM"))
                pso = ph2.enter_context(tc.tile_pool(name="pso", bufs=1, space="PSUM"))

                # qb-outer so each q-block's proj can interleave into the
                # next q-block's attention
                units = [(h, qb) for qb in range(NQB) for h in range(HLOC)]
                nqt_ctr = [0]

                def emit_head_qt(unit, qt):
                    """scores + softmax for one q-tile; returns normalized attn."""
                    h, qb = unit
                    qh, kh = qk_sb[2 * h], qk_sb[2 * h + 1]
                    qti = qb * 4 + qt
                    sc = [
                        pss.tile([128, w], F32, name="ps_s", tag="ps_s")
                        for w in SCW
                    ]
                    off = 0
                    for i, w in enumerate(SCW):
                        for j in range(w // 512):
                            nc.tensor.matmul(
                                sc[i][:, j * 512:(j + 1) * 512],
                                qh[:, qti * 128:(qti + 1) * 128],
                                kh[:, off:off + 512],
                                start=True,
                                stop=True,
                            )
                            off += 512
                    mAB = stats.tile([128, NSC], F32, name="mAB", tag="mAB")
                    for i in range(NSC):
                        nc.vector.tensor_reduce(
                            mAB[:, i:i + 1], sc[i][:], axis=AX.X, op=OP.max
                        )
                    negm = stats.tile([128, 1], F32, name="negm", tag="negm")
                    nc.vector.tensor_reduce(
                        negm[:], mAB[:], axis=AX.X, op=OP.max, negate=True
                    )
                    attn_t = attn_pool.tile([128, S], BF16, name="attn_t", tag="attn_t")
                    sm = stats.tile([128, NSC], F32, name="sm", tag="sm")
                    off = 0
                    for i, w in enumerate(SCW):
                        nc.scalar.activation(
                            attn_t[:, off:off + w],
                            sc[i][:],
                            ACTF.Exp,
                            bias=negm[:],
                            scale=1.0,
                            accum_out=sm[:, i:i + 1],
                        )
                        off += w
                    sumx = stats.tile([128, 1], F32, name="sumx", tag="sumx")
                    nc.vector.tensor_reduce(sumx[:], sm[:], axis=AX.X, op=OP.add)
                    recip = stats.tile([128, 1], F32, name="recip", tag="recip")
                    nc.vector.reciprocal(recip[:], sumx[:])
                    # balance DVE vs Act: every 4th normalize runs on Act
                    if nqt_ctr[0] % 4 == 3:
                        nc.scalar.mul(attn_t[:], attn_t[:], recip[:])
                    else:
                        nc.vector.tensor_scalar_mul(attn_t[:], attn_t[:], recip[:])
                    nqt_ctr[0] += 1
                    return attn_t

                def tail_chunks(state):
                    """Closures: transposes + (lagged) AV for a completed unit,
                    one kt-pair per chunk. AV for pair k is emitted in chunk
                    k+1 so the PE never waits on the pt->at copy it depends on."""
                    h, qb, attns = state
                    nktp = NKT // 2
                    ps_o_box = [None]
                    at_tiles = [None] * nktp

                    def emit_av(ktp):
                        for half in range(2):
                            kt = 2 * ktp + half
                            nc.tensor.matmul(
                                ps_o_box[0][:],
                                v_sb[kt][:, h * 128:(h + 1) * 128],
                                at_tiles[ktp][:, half * 512:(half + 1) * 512],
                                start=(kt == 0),
                                stop=(kt == NKT - 1),
                            )

                    def mk_ktp(ktp):
                        def go():
                            if ktp == 0:
                                ps_o_box[0] = pso.tile(
                                    [128, 512], F32, name="ps_o", tag="ps_o"
                                )
                            pt = pst.tile([128, 1024], BF16, name="ps_t", tag="ps_t")
                            for half in range(2):
                                kt = 2 * ktp + half
                                for qt in range(4):
                                    nc.tensor.transpose(
                                        pt[:, half * 512 + qt * 128:
                                           half * 512 + (qt + 1) * 128],
                                        attns[qt][:, kt * 128:(kt + 1) * 128],
                                        ident[:],
                                    )
                            at = attnT_pool.tile([128, 1024], BF16, name="at", tag="at")
                            at_tiles[ktp] = at
                            # split copy: halves on DVE and Act in parallel
                            nc.vector.tensor_copy(at[:, 0:512], pt[:, 0:512])
                            nc.scalar.copy(at[:, 512:1024], pt[:, 512:1024])
                            if ktp > 0:
                                emit_av(ktp - 1)
                        return go

                    def fin():
                        emit_av(nktp - 1)
                        nc.scalar.copy(
                            oT_sb[h][:, qb * 512:(qb + 1) * 512], ps_o_box[0][:]
                        )

                    return [mk_ktp(k) for k in range(nktp)] + [fin]

                pending = []  # tail + proj chunks awaiting interleave
                for iu, u in enumerate(units):
                    nq = 4
                    for qt in range(nq):
                        attn_t = emit_head_qt(u, qt)
                        if qt == 0:
                            attns = []
                        attns.append(attn_t)
                        # interleave pending work: spread chunks across qts
                        take = (len(pending) + (nq - 1 - qt)) // (nq - qt)
                        for _ in range(take):
                            pending.pop(0)()
                    pending = pending + tail_chunks((u[0], u[1], attns))
                for ch in pending:
                    ch()

            # ---------------- Phase 3: proj (partial) ----------------
            with ExitStack() as ph3:
                y_pool = ph3.enter_context(tc.tile_pool(name="y_pool", bufs=6))
                psy = ph3.enter_context(tc.tile_pool(name="psy", bufs=4, space="PSUM"))

                nyc = [0]
                for eb in range(NEB):
                    for qti in range(NST):
                        ps_y = psy.tile([128, 512], F32, name="ps_y", tag="ps_y")
                        for hh in range(HLOC):
                            nc.tensor.matmul(
                                ps_y[:],
                                oT_sb[hh][:, qti * 128:(qti + 1) * 128],
                                wp_sb[hh][:, eb * 512:(eb + 1) * 512],
                                start=(hh == 0),
                                stop=(hh == HLOC - 1),
                            )
                        y_t = y_pool.tile([128, 512], F16, name="y_t", tag="y_t")
                        if nyc[0] % 2 == 0:
                            nc.vector.tensor_copy(y_t[:], ps_y[:])
                        else:
                            nc.scalar.copy(y_t[:], ps_y[:])
                        nyc[0] += 1
                        nc.sync.dma_start(
                            y_p[qti * 128:(qti + 1) * 128, eb * 512:(eb + 1) * 512],
                            y_t[:],
                        )

    _split_multi_waits(nc)
    return nc


# ---------------------------------------------------------------------------
# Host-side sharding / gather
# ---------------------------------------------------------------------------

def _prep_in_maps(query, Wqkv, bqkv, Wproj, S=S_FULL, HLOC=HLOC_FULL, n_cores=N_CORES):
    scale = np.float32(HD ** 0.5)
    groups = n_cores // B
    in_maps = []
    xt_cache = {}
    for c in range(n_cores):
        b, g = c // groups, c % groups
        heads = [g * HLOC + hh for hh in range(HLOC)]
        if b not in xt_cache:
            xt_cache[b] = np.ascontiguousarray(
                query[b][:S].T.astype(np.float16)
            )
        NFT = 2 * HLOC
        wqk = np.empty((NFT, 128, E), dtype=np.float16)
        bqk = np.empty((NFT * 128,), dtype=np.float32)
        wv = np.empty((E, HLOC * 128), dtype=np.float16)
        bv = np.empty((HLOC * 128,), dtype=np.float32)
        wp = np.empty((HLOC * 128, E), dtype=np.float16)
        for hh, hd_ in enumerate(heads):
            base = hd_ * (3 * HD)
            wq = Wqkv[base:base + HD, :] * scale          # [128, E]
            wk = Wqkv[base + HD:base + 2 * HD, :]
            wvh = Wqkv[base + 2 * HD:base + 3 * HD, :]
            # [E,128] -> chunked [128, E] layout: arr[p, c*128+j] = W.T[c*128+p, j]
            wqk[2 * hh] = (
                wq.T.reshape(E // 128, 128, HD).transpose(1, 0, 2).reshape(128, E)
            ).astype(np.float16)
            wqk[2 * hh + 1] = (
                wk.T.reshape(E // 128, 128, HD).transpose(1, 0, 2).reshape(128, E)
            ).astype(np.float16)
            bqk[2 * hh * 128:(2 * hh + 1) * 128] = bqkv[base:base + HD] * scale
            bqk[(2 * hh + 1) * 128:(2 * hh + 2) * 128] = bqkv[base + HD:base + 2 * HD]
            wv[:, hh * 128:(hh + 1) * 128] = wvh.T.astype(np.float16)
            bv[hh * 128:(hh + 1) * 128] = bqkv[base + 2 * HD:base + 3 * HD]
            wp[hh * 128:(hh + 1) * 128, :] = Wproj[:, hd_ * HD:(hd_ + 1) * HD].T.astype(
                np.float16
            )
        in_maps.append(
            {
                "xt": xt_cache[b],
                "wqk": np.ascontiguousarray(wqk),
                "wv": np.ascontiguousarray(wv),
                "bqk": bqk.reshape(1, -1).astype(np.float16),
                "bv": bv.reshape(1, -1).astype(np.float16),
                "wp": np.ascontiguousarray(wp),
            }
        )
    return in_maps


_CACHE = {}


def _get_program(S=S_FULL, HLOC=HLOC_FULL, has_bias=True):
    key = (S, HLOC, has_bias)
    if key not in _CACHE:
        _CACHE[key] = build_program(S, HLOC, has_bias=has_bias)
    return _CACHE[key]


def run(query, Wqkv, bqkv, Wproj, bproj, trace=False, S=S_FULL, HLOC=HLOC_FULL,
        n_cores=N_CORES):
    from concourse.bass_utils import run_bass_kernel_spmd

    has_bias = bool(np.any(bqkv))
    nc = _get_program(S, HLOC, has_bias=has_bias)
    in_maps = _prep_in_maps(query, Wqkv, bqkv, Wproj, S=S, HLOC=HLOC, n_cores=n_cores)
    res = run_bass_kernel_spmd(
        nc, in_maps, core_ids=list(range(n_cores)), trace=trace
    )
    groups = n_cores // B
    out = np.zeros((B, S, E), dtype=np.float32)
    for c in range(n_cores):
        out[c // groups] += res.results[c]["y"].astype(np.float32)
    out += bproj.astype(np.float32)
    return out, res


def kernel(**inputs):
    out, _ = run(
        np.asarray(inputs["query"], dtype=np.float32),
        np.asarray(inputs["Wqkv"], dtype=np.float32),
        np.asarray(inputs["bqkv"], dtype=np.float32),
        np.asarray(inputs["Wproj"], dtype=np.float32),
        np.asarray(inputs["bproj"], dtype=np.float32),
        trace=False,
    )
    return out

